# revision 1
# baseline (speedup 1.0000x reference)
"""Trainium2 Bass kernel for nn_InpaintContextAttentionUnit.

Per-sample computation (B=8 samples -> 1 per NeuronCore):
  fm [512,512,16] -> avgpool(64x2) -> pooled [8,256,16]
  -> two masked 3x3 convs (middle row / middle col of kernel zeroed) + bias + relu
  -> bilinear upsample back to [512,512,16] (separable; half-pixel centers, edge clamp)
  -> out [512,512,48] = concat(fm, fm - row_up, fm - col_up)

Design (v2 — single fm read, contiguous DVE paths, DMA-overlap-first):
  - fm is loaded from HBM exactly once (4x 4 MiB fp32 HWDGE loads); ACT casts each
    tile into a persistent bf16 copy used by pooling AND by the pass-B subtract /
    passthrough (bf16 roundtrip error ~2^-9 rel, well under the 2e-2 gate)
  - pooling: PE matmul with a [128,2] block-mean matrix; rhs kept (xp, c)-inner-
    contiguous (strided rhs costs ~5 cyc/col on PE); contiguous CAST to bf16
    stage, SBUF-hop to an assembled ncw [8n, (xp c)]; the c<->w free transpose
    runs ONCE on 8 partitions (split DVE/ACT halves) instead of 4x on 2
    partitions (v1: 18us each)
  - conv: per (branch, n-pair chunk): zero-init matmul + ~6 accumulating
    [16c,16f]x[16c,<=512] matmuls in PSUM; relu+bias on ACT; per-branch DRAM
    bounce so branch-0 W-upsample (DVE) overlaps branch-1 conv (PE)
  - W-upsample (x2): 2 scalar_tensor_tensor ops per branch computing
    pad[k]*1/3 + pad[k+1] (the 0.75 factor is folded into the host hup matrix,
    exact in bf16), written (f, x)-major (x-inner iteration is the fast STT path)
  - H-upsample (x64): PE matmuls rw[8n, (f, x)] with host-built 0.75*HUp matrix
  - combine: DVE subtract (bf16 fm - psum) + ACT copy into interleaved
    [y, x, 48ch] staging tiles (bufs=4), contiguous 3 MiB DMAs out
All constant matrices are precomputed on host and passed as extra inputs.
"""

import numpy as np
import ml_dtypes

H, W, C, F = 512, 512, 16, 16
NPOOL = 8
WP = W // 2  # 256
CH_OUT = 3 * C  # 48

_cache = {}


def _host_consts(kernel, bias):
    """Build host-side constant matrices (bf16 for the PE-side constants)."""
    bf = ml_dtypes.bfloat16
    # pooling weights: [128, 2], 1/128 (exact in bf16) where row block matches
    poolw = np.zeros((128, 2), np.float32)
    poolw[:64, 0] = 1.0 / 128.0
    poolw[64:, 1] = 1.0 / 128.0
    # H-upsample matrix: hup[n, y] = weight of pooled row n for output row y,
    # scaled by 0.75 (the W-upsample major tap; k/64*0.75 = 3k/256 exact in bf16)
    hup = np.zeros((NPOOL, H), np.float32)
    scale = H // NPOOL
    for y in range(H):
        yf = (y + 0.5) / scale - 0.5
        i0 = int(np.floor(yf))
        w = yf - i0
        hup[min(max(i0, 0), NPOOL - 1), y] += 1.0 - w
        hup[min(max(i0 + 1, 0), NPOOL - 1), y] += w
    hup *= 0.75
    hup2 = np.zeros((40, H), np.float32)
    hup2[0:8] = hup
    hup2[32:40] = hup  # col-branch copy at base partition 32; rows 8-15 stay zero
    # conv taps, stacked over dwp on 48 partitions (partition 16g+c holds the
    # dwp=g-1 shifted pooled copy). One matmul slot per (branch, dn):
    #   branch 0 (row conv): kernel[dn+1, dwp+1]; slots 0,1 for dn=-1,+1
    #   branch 1 (col conv): kernel[dwp+1, dn+1], dwp=0 block zero; slots 2-4
    kt = np.zeros((48, 5 * 16), np.float32)  # [(g,c), slot*16+f]
    for g in range(3):
        for s, dn in enumerate((-1, 1)):
            kt[16 * g:16 * (g + 1), s * 16:(s + 1) * 16] = kernel[dn + 1, g]
        for s, dn in enumerate((-1, 0, 1)):
            if g != 1:
                kt[16 * g:16 * (g + 1), (2 + s) * 16:(3 + s) * 16] = \
                    kernel[g, dn + 1]
    bias2 = np.ascontiguousarray(bias.reshape(16, 1)).astype(np.float32)
    return (poolw.astype(bf), hup2.astype(bf), kt.astype(bf), bias2, None, None)



def _build_program(compile=True):
    import concourse.bass as bass
    import concourse.bacc as bacc
    import concourse.mybir as mybir
    import concourse.tile as tile

    dt = mybir.dt.float32
    db = mybir.dt.bfloat16
    nc = bacc.Bacc()

    fm_d = nc.declare_dram_parameter("feature_map", [H, W, C], dt, isOutput=False)
    poolw_d = nc.declare_dram_parameter("poolw", [128, 2], db, isOutput=False)
    hup_d = nc.declare_dram_parameter("hup", [40, H], db, isOutput=False)
    ktaps_d = nc.declare_dram_parameter("ktaps", [48, 80], db, isOutput=False)
    bias_d = nc.declare_dram_parameter("bias2", [16, 1], dt, isOutput=False)
    out_d = nc.declare_dram_parameter("out", [H, W, CH_OUT], dt, isOutput=True)

    # matmul slots per branch: (slot, dn)
    slots_by_branch = [[(0, -1), (1, 1)], [(2, -1), (3, 0), (4, 1)]]

    with tile.TileContext(nc) as tc:
        with (
            tc.tile_pool(name="consts", bufs=1) as cpool,
            tc.tile_pool(name="persist", bufs=1) as ppool,
            tc.tile_pool(name="mid", bufs=1) as mpool,
            tc.tile_pool(name="dram", bufs=1, space="DRAM") as dpool,
            tc.tile_pool(name="psall", bufs=1, space="PSUM") as psall,
        ):
            poolw_t = cpool.tile([128, 2], db)
            hup_t = cpool.tile([40, H], db)
            ktaps_t = cpool.tile([48, 80], db)
            bias_t = cpool.tile([16, 1], dt)

            # persistent bf16 fm copy: [128, (4 t, 512 x, 16 c)]
            fmb_t = ppool.tile([128, 4 * W * C], db)
            # rw [40, (16 f, 512 x)] bf16: partitions 0-7 row-branch, 32-39 col
            rw_t = ppool.tile([40, 16 * W], db)

            # mid-lived pass-A tiles (alive into the overlapped pass-B phase)
            tpad_t = mpool.tile([48, 10 * 258], db)
            t48 = tpad_t[:].rearrange("p (n w) -> p n w", w=258)
            conv_t = mpool.tile([16, 2 * NPOOL * WP], db)
            rop_t = mpool.tile([40, 16 * 258], db)
            rop3 = rop_t[:].rearrange("p (f w) -> p f w", w=258)
            rwv = rw_t[:].rearrange("p (f xp par) -> p f par xp", par=2, xp=WP)

            ncw_dram = dpool.tile([NPOOL, 16 * 258], db)
            nd3 = ncw_dram[:].rearrange("n (c w) -> n c w", w=258)
            ncwd3 = ncw_dram[:].rearrange("n (c w) -> c n w", w=258)
            conv_dram = dpool.tile([16, 2 * NPOOL * WP], db)
            cd4 = conv_dram[:].rearrange("f (b n w) -> b n f w", b=2, n=NPOOL)
            zsrc = hup_d[8:16, 0:16]  # [8, 16] zeros

            # zero-fill: t48 halo rows; rop rows 5-7 are read (edge copy /
            # W-up) before half-1 writes them — NaN garbage would poison the
            # zero-weighted hup products
            nc.vector.memset(tpad_t[:], 0.0)
            nc.vector.memset(rop_t[:], 0.0)

            # pooling rhs view of fmb: (t, xp, par, c) — c-inner contiguous
            fmr = fmb_t[:].rearrange(
                "p (t xp par c) -> p t xp par c", t=4, par=2, c=16)

            def conv_unit(b, n0, nn):
                # conv rows n0..n0+nn; dwp taps contracted via the
                # 48-partition stack, one accumulating matmul per dn
                ps = psall.tile([16, 2 * WP], dt, tag="conv", bufs=2,
                                name=f"psc{b}{n0}")
                slots = slots_by_branch[b]
                for k, (sl, dn) in enumerate(slots):
                    nc.tensor.matmul(
                        ps[:, 0:nn * WP],
                        ktaps_t[:, sl * 16:(sl + 1) * 16],
                        t48[:, n0 + dn + 1:n0 + dn + 1 + nn, 1:257],
                        start=(k == 0), stop=(k == len(slots) - 1),
                    )
                nc.scalar.activation(
                    out=conv_t[:, (b * NPOOL + n0) * WP:
                               (b * NPOOL + n0 + nn) * WP],
                    in_=ps[:, 0:nn * WP],
                    func=mybir.ActivationFunctionType.Relu,
                    bias=bias_t[:, 0:1],
                )

            def tail_half(b, hf):
                # bounce the half's conv rows to [(b,n) parts, (f, wp)]
                nlo, nhi = (0, 5) if hf == 0 else (5, 8)
                pg = 32 * b
                nc.sync.dma_start(
                    out=conv_dram[:, (b * NPOOL + nlo) * WP:
                                  (b * NPOOL + nhi) * WP],
                    in_=conv_t[:, (b * NPOOL + nlo) * WP:
                               (b * NPOOL + nhi) * WP])
                nc.sync.dma_start(
                    out=rop3[pg + nlo:pg + nhi, :, 1:257],
                    in_=cd4[b][nlo:nhi])

            def wup_round():
                # W-upsample into (f, x)-major rw; 0.75 folded into hup:
                #   rw[2k] = pad[k]/3 + pad[k+1]; rw[2k+1] = pad[k+2]/3 + pad[k+1]
                # Both branches in single 40-partition ops: DVE time scales
                # with per-partition elements, so spanning rows 0:40 (rows
                # 8-31 are memset zeros feeding unused rw rows) halves the
                # op count vs per-branch 8-partition ops.
                nc.vector.tensor_copy(
                    rop3[0:40, :, 0:1], rop3[0:40, :, 1:2])
                nc.vector.tensor_copy(
                    rop3[0:40, :, 257:258], rop3[0:40, :, 256:257])
                third = 1.0 / 3.0
                nc.vector.scalar_tensor_tensor(
                    out=rwv[0:40, :, 0, :],
                    in0=rop3[0:40, :, 0:256],
                    scalar=third,
                    in1=rop3[0:40, :, 1:257],
                    op0=mybir.AluOpType.mult,
                    op1=mybir.AluOpType.add,
                )
                nc.vector.scalar_tensor_tensor(
                    out=rwv[0:40, :, 1, :],
                    in0=rop3[0:40, :, 2:258],
                    scalar=third,
                    in1=rop3[0:40, :, 1:257],
                    op0=mybir.AluOpType.mult,
                    op1=mybir.AluOpType.add,
                )

            # ---------------- pass A: per-tile load/pool/bounce ----------------
            with tc.tile_pool(name="passA", bufs=1) as apool:
                for t in range(4):
                    fmfs = []
                    for h in range(2):
                        fmf = apool.tile([128, W * C // 2], dt,
                                         tag="fmf", bufs=3)
                        fmf3 = fmf[:].rearrange("p (x c) -> p x c", c=C)
                        nc.sync.dma_start(
                            out=fmf3,
                            in_=fm_d[128 * t:128 * (t + 1),
                                     256 * h:256 * (h + 1)])
                        fmfs.append(fmf)
                        if t == 0 and h == 0:
                            nc.sync.dma_start(out=poolw_t[:], in_=poolw_d[:])
                            nc.sync.dma_start(out=hup_t[:], in_=hup_d[:])
                            nc.sync.dma_start(out=ktaps_t[:], in_=ktaps_d[:])
                            nc.sync.dma_start(out=bias_t[:], in_=bias_d[:])
                            nc.sync.dma_start(out=nd3[:, :, 0:1], in_=zsrc)
                            nc.sync.dma_start(out=nd3[:, :, 257:258], in_=zsrc)
                    # bf16 casts: ACT takes half 0, DVE half 1
                    half = W * C // 2
                    nc.scalar.activation(
                        out=fmb_t[:, t * W * C:t * W * C + half],
                        in_=fmfs[0][:],
                        func=mybir.ActivationFunctionType.Copy)
                    nc.vector.tensor_copy(
                        fmb_t[:, t * W * C + half:(t + 1) * W * C],
                        fmfs[1][:])
                    # H-pool (y->n) + W-pair add; (xp, c)-major psum in 1-bank
                    # eighths, f32->bf16 CAST alternating DVE/ACT
                    stage = apool.tile([2, WP * 16], db, tag="stage", bufs=1)
                    for e in range(8):
                        ps = psall.tile([2, 512], dt, tag="pool", bufs=2,
                                        name=f"psp{t}{e}")
                        for par in range(2):
                            nc.tensor.matmul(
                                ps[:], poolw_t[:],
                                fmr[:, t, 32 * e:32 * (e + 1), par, :],
                                start=(par == 0), stop=(par == 1),
                            )
                        dst = stage[:, 512 * e:512 * (e + 1)]
                        if e < 2:
                            nc.vector.tensor_copy(dst, ps[:])
                        else:
                            nc.scalar.activation(
                                out=dst, in_=ps[:],
                                func=mybir.ActivationFunctionType.Copy)
                    # free-dim transpose (xp, c) -> (c, w), split DVE/ACT
                    stageT = apool.tile([2, WP * 16], db, tag="stageT", bufs=1)
                    st_cx = stage[:].rearrange("p (x c) -> p c x", c=16)
                    stT3 = stageT[:].rearrange("p (c x) -> p c x", x=WP)
                    nc.vector.tensor_copy(stT3[:, 0:8, :], st_cx[:, 0:8, :])
                    nc.vector.tensor_copy(stT3[:, 8:16, :], st_cx[:, 8:16, :])
                    # bounce to DRAM; read back 3 dwp-shifted copies with
                    # c on partitions (n rows shifted +1 for the zero halo)
                    nc.sync.dma_start(
                        out=nd3[2 * t:2 * t + 2, :, 1:257], in_=stT3)
                    for g in range(3):
                        nc.sync.dma_start(
                            out=t48[16 * g:16 * (g + 1),
                                    2 * t + 1:2 * t + 3, 1:257],
                            in_=ncwd3[:, 2 * t:2 * t + 2, g:g + 256])
                    # conv units whose tpad rows are complete:
                    if t == 1:
                        conv_unit(0, 0, 2)
                        conv_unit(1, 0, 2)
                    elif t == 2:
                        # half-0 completes: conv n2..4, bounce, W-up round 0
                        conv_unit(0, 2, 2)
                        conv_unit(0, 4, 1)
                        tail_half(0, 0)
                        conv_unit(1, 2, 2)
                        conv_unit(1, 4, 1)
                        tail_half(1, 0)
                        wup_round()

            # half-1 conv (needs tile-3 tpad; mpool tiles outlive apool)
            conv_unit(0, 5, 1)
            conv_unit(0, 6, 2)
            tail_half(0, 1)
            conv_unit(1, 5, 1)
            conv_unit(1, 6, 2)
            tail_half(1, 1)

            # ------------- pass B: H-upsample + combine + store -------------
            # tiles 0-1 only read hup rows n<=4 (zero weight for n>4), valid
            # after W-up round 0; round 1 (after half-1 conv) re-writes rw
            # with identical n0-4 values plus valid n5-7 for tiles 2-3
            with tc.tile_pool(name="passB", bufs=1) as bpool:
                fmb4 = fmb_t[:].rearrange("p (t x c) -> p t x c", t=4, c=16)
                rwx = rw_t[:].rearrange("p (f x) -> p f x", x=W)

                def passB_tile(t):
                    for q in range(4):
                        outq = bpool.tile([128, 128 * CH_OUT], dt,
                                          tag="outq", bufs=3, name=f"oq{t}{q}")
                        outq3 = outq[:].rearrange("p (x ch) -> p x ch",
                                                  ch=CH_OUT)
                        fmq = fmb4[:, t, 128 * q:128 * (q + 1), :]
                        nc.scalar.activation(
                            out=outq3[:, :, 0:16], in_=fmq,
                            func=mybir.ActivationFunctionType.Copy,
                        )
                        for b in range(2):
                            pg = 32 * b
                            lhsT = hup_t[pg:pg + 8, 128 * t:128 * (t + 1)]
                            for fh in range(2):
                                ps = psall.tile([128, 1024], dt, tag="up",
                                                bufs=2, name=f"psu{t}{q}{b}{fh}")
                                for i in range(2):
                                    nc.tensor.matmul(
                                        ps[:, 512 * i:512 * (i + 1)],
                                        lhsT,
                                        rwx[pg:pg + 8,
                                            8 * fh + 4 * i:8 * fh + 4 * (i + 1),
                                            128 * q:128 * (q + 1)],
                                        start=True, stop=True,
                                    )
                                psx = ps[:].rearrange("p (f x) -> p x f", x=128)
                                nc.vector.tensor_sub(
                                    outq3[:, :, 16 * (b + 1) + 8 * fh:
                                          16 * (b + 1) + 8 * (fh + 1)],
                                    fmq[:, :, 8 * fh:8 * (fh + 1)], psx)
                        nc.sync.dma_start(
                            out=out_d[128 * t:128 * (t + 1),
                                      128 * q:128 * (q + 1), :],
                            in_=outq3,
                        )

                passB_tile(0)
                wup_round()  # round 1: fills n5-7 (n0-4 recomputed, identical)
                passB_tile(1)
                passB_tile(2)
                passB_tile(3)
    if compile:
        nc.compile()
    return nc


def _get_program():
    if "nc" not in _cache:
        _cache["nc"] = _build_program()
    return _cache["nc"]


def kernel(feature_map, kernel, bias):
    from concourse.bass_utils import run_bass_kernel_spmd

    feature_map = np.ascontiguousarray(feature_map, dtype=np.float32)
    kernel = np.ascontiguousarray(kernel, dtype=np.float32)
    bias = np.ascontiguousarray(bias, dtype=np.float32)
    B = feature_map.shape[0]
    assert B == 8

    poolw, hup, kt, bias2, _, _ = _host_consts(kernel, bias)
    nc = _get_program()
    in_maps = [
        {
            "feature_map": feature_map[b],
            "poolw": poolw,
            "hup": hup,
            "ktaps": kt,
            "bias2": bias2,
        }
        for b in range(B)
    ]
    res = run_bass_kernel_spmd(nc, in_maps, list(range(B)))
    out = np.stack([res.results[b]["out"] for b in range(B)])
    return out

